# revision 3
# baseline (speedup 1.0000x reference)
"""Distributed GCN (DMoN front-end) kernel for 8 Trainium2 NeuronCores.

Strategy (matches the sharding hint):
  - Nodes are partitioned contiguously across the 8 cores; each core owns its
    nodes' incident (incoming) edges, grouped by destination block of 128.
  - spmm is computed as a sequence of one-hot "segment matmuls": for each tile
    of 128 edges (sorted by destination), build S[e, d] = w_e * (rel_dst[e]==d)
    on the vector engine and accumulate P += S^T @ gathered_rows on the PE,
    where gathered_rows come from an indirect-DMA gather of source-node rows.
  - The linear transform is applied AFTER aggregation (linearity):
        selu(skip*(xW+b) + spmm(xW+b)) == selu((x_own + spmm_raw(x))W + (deg_w
        + skip)*b)            (skip == 1 in this model)
    so the gather tables are the raw features (x, then h1) — no transformed
    table ever has to be materialized.
  - A per-row "ones" column is baked into the gather table so the same matmul
    chain also produces deg_w (sum of incident edge weights) for the bias term.
  - Between the two GCN layers, h1 is AllGathered across the 8 cores so every
    core can gather any source row of h1.
"""

import math

import numpy as np

C = 8            # cores
D = 128          # feature dim
KCL = 16         # clusters
ROWW = 132       # gather-table row width: D feats + ones col + 3 pad (16B align)
GRP = 4          # dst blocks per PSUM epilogue group
SELU_ALPHA = 1.6732632423543772
SELU_SCALE = 1.0507009873554805

_CACHE = {}


# ----------------------------------------------------------------------------
# host-side preprocessing (pure index manipulation + layout)
# ----------------------------------------------------------------------------

def preprocess(x, edge_index, edge_weight):
    N = x.shape[0]
    E = edge_index.shape[1]
    npc_real = N // C
    assert N % C == 0
    B = math.ceil(npc_real / 128)
    NPC = 128 * B
    Npad = C * NPC

    dst = edge_index[0].astype(np.int64)
    src = edge_index[1].astype(np.int64)
    core = dst // npc_real
    dst_loc = dst - core * npc_real
    blk = dst_loc // 128
    rel = (dst_loc % 128).astype(np.float32)
    gblk = core * B + blk

    counts = np.bincount(gblk, minlength=C * B)
    TPB = max(1, int(math.ceil(counts.max() / 128)))
    T = B * TPB

    order = np.argsort(gblk, kind="stable")
    starts = np.zeros(C * B + 1, np.int64)
    starts[1:] = np.cumsum(counts)
    gs = gblk[order]
    pos = np.arange(E, dtype=np.int64) - starts[gs]

    core_s = core[order]
    blk_s = blk[order]
    col = blk_s * TPB + pos // 128
    row = pos % 128

    src_s = src[order]
    src_pad = (src_s // npc_real) * NPC + (src_s % npc_real)

    SRC = np.zeros((C, 128, T), np.int32)
    REL = np.zeros((C, 128, T), np.float32)
    WGT = np.zeros((C, 128, T), np.float32)
    SRC[core_s, row, col] = src_pad.astype(np.int32)
    REL[core_s, row, col] = rel[order]
    WGT[core_s, row, col] = np.asarray(edge_weight, np.float32)[order]

    # padded gather table for layer 1, with the ones column at index D
    xt = np.zeros((C, NPC, ROWW), np.float32)
    xt[:, :npc_real, :D] = np.asarray(x, np.float32).reshape(C, npc_real, D)
    xt[:, :, D] = 1.0
    xfull = np.ascontiguousarray(xt.reshape(Npad, ROWW))
    xown = np.ascontiguousarray(xt[:, :, :D])          # [C, NPC, D]

    meta = dict(N=N, E=E, npc_real=npc_real, B=B, NPC=NPC, Npad=Npad,
                TPB=TPB, T=T)
    return meta, SRC, REL, WGT, xfull, xown


# ----------------------------------------------------------------------------
# bass program
# ----------------------------------------------------------------------------

def build_program(B, TPB, NPC, Npad):
    import concourse.bass as bass
    import concourse.bacc as bacc
    import concourse.mybir as mybir
    from concourse.tile import TileContext
    from concourse.masks import make_identity

    F32 = mybir.dt.float32
    I32 = mybir.dt.int32
    OP = mybir.AluOpType
    ACT = mybir.ActivationFunctionType
    T = B * TPB
    SA = SELU_SCALE * SELU_ALPHA

    nc = bacc.Bacc("TRN2", target_bir_lowering=False, debug=False,
                   num_devices=C)

    xfull = nc.dram_tensor("xfull", [Npad, ROWW], F32, kind="ExternalInput")
    xown = nc.dram_tensor("xown", [NPC, D], F32, kind="ExternalInput")
    srci = nc.dram_tensor("srci", [128, T], I32, kind="ExternalInput")
    reli = nc.dram_tensor("reli", [128, T], F32, kind="ExternalInput")
    wgti = nc.dram_tensor("wgti", [128, T], F32, kind="ExternalInput")
    w1 = nc.dram_tensor("w1", [D, D], F32, kind="ExternalInput")
    w2 = nc.dram_tensor("w2", [D, D], F32, kind="ExternalInput")
    wa = nc.dram_tensor("wa", [D, KCL], F32, kind="ExternalInput")
    b1r = nc.dram_tensor("b1r", [128, D], F32, kind="ExternalInput")
    b2r = nc.dram_tensor("b2r", [128, D], F32, kind="ExternalInput")
    bar = nc.dram_tensor("bar", [128, KCL], F32, kind="ExternalInput")
    iot = nc.dram_tensor("iot", [128, D], F32, kind="ExternalInput")

    h1own = nc.dram_tensor("h1own", [NPC, ROWW], F32)
    h1full = nc.dram_tensor("h1full", [Npad, ROWW], F32, addr_space="Shared")
    outp = nc.dram_tensor("assign", [NPC, KCL], F32, kind="ExternalOutput")

    with TileContext(nc) as tc:
        with tc.tile_pool(name="const", bufs=1) as const, \
             tc.tile_pool(name="g", bufs=12) as gpool, \
             tc.tile_pool(name="s", bufs=8) as spool, \
             tc.tile_pool(name="z", bufs=4) as zpool, \
             tc.tile_pool(name="dg", bufs=8) as dpool, \
             tc.tile_pool(name="zt", bufs=4) as ztpool, \
             tc.tile_pool(name="bt", bufs=2) as btpool, \
             tc.tile_pool(name="ep", bufs=2) as epool, \
             tc.tile_pool(name="xo", bufs=4) as xopool, \
             tc.tile_pool(name="l3", bufs=3) as l3pool, \
             tc.tile_pool(name="pp", bufs=3, space="PSUM") as ppool, \
             tc.tile_pool(name="qp", bufs=2, space="PSUM") as qpool, \
             tc.tile_pool(name="tp", bufs=2, space="PSUM") as tpool:

            # ---- persistent state ----
            idx_sb = const.tile([128, T], I32)
            nc.sync.dma_start(out=idx_sb[:], in_=srci[:, :])
            rel_sb = const.tile([128, T], F32)
            nc.sync.dma_start(out=rel_sb[:], in_=reli[:, :])
            wgt_sb = const.tile([128, T], F32)
            nc.sync.dma_start(out=wgt_sb[:], in_=wgti[:, :])
            iota_sb = const.tile([128, D], F32)
            nc.sync.dma_start(out=iota_sb[:], in_=iot[:, :])
            w1_sb = const.tile([128, D], F32)
            nc.sync.dma_start(out=w1_sb[:], in_=w1[:, :])
            w2_sb = const.tile([128, D], F32)
            nc.sync.dma_start(out=w2_sb[:], in_=w2[:, :])
            wa_sb = const.tile([128, KCL], F32)
            nc.sync.dma_start(out=wa_sb[:], in_=wa[:, :])
            b1r_sb = const.tile([128, D], F32)
            nc.sync.dma_start(out=b1r_sb[:], in_=b1r[:, :])
            b2r_sb = const.tile([128, D], F32)
            nc.sync.dma_start(out=b2r_sb[:], in_=b2r[:, :])
            bar_sb = const.tile([128, KCL], F32)
            nc.sync.dma_start(out=bar_sb[:], in_=bar[:, :])
            ident = const.tile([128, 128], F32)
            make_identity(nc, ident[:])

            h1_sb = const.tile([128, B, ROWW], F32)
            nc.vector.memset(h1_sb[:, :, D:ROWW], 1.0)
            h2_sb = const.tile([128, B, D], F32)

            def layer(table, feats_own, W_sb, brep_sb, h_write, dram_out):
                for g0 in range(0, B, GRP):
                    gb = min(GRP, B - g0)
                    w = gb * 128
                    q_ps = qpool.tile([128, GRP * 128], F32, tag="q")
                    bt = btpool.tile([128, GRP * 128], F32, tag="bt")
                    for j in range(gb):
                        b = g0 + j
                        P = ppool.tile([128, ROWW], F32, tag="p")
                        for t in range(TPB):
                            tcol = b * TPB + t
                            gt = gpool.tile([128, ROWW], F32, tag="g")
                            nc.gpsimd.indirect_dma_start(
                                out=gt[:, :], out_offset=None, in_=table[:, :],
                                in_offset=bass.IndirectOffsetOnAxis(
                                    ap=idx_sb[:, tcol:tcol + 1], axis=0))
                            St = spool.tile([128, 128], F32, tag="s")
                            nc.vector.tensor_scalar(
                                out=St[:], in0=iota_sb[:],
                                scalar1=rel_sb[:, tcol:tcol + 1],
                                scalar2=wgt_sb[:, tcol:tcol + 1],
                                op0=OP.is_equal, op1=OP.mult)
                            nc.tensor.matmul(
                                P[:, 0:D + 1], lhsT=St[:], rhs=gt[:, 0:D + 1],
                                start=(t == 0), stop=(t == TPB - 1))
                        fo = feats_own(b)
                        Z = zpool.tile([128, D], F32, tag="z")
                        nc.vector.tensor_tensor(out=Z[:], in0=P[:, 0:D],
                                                in1=fo, op=OP.add)
                        dg = dpool.tile([128, 1], F32, tag="dg")
                        nc.vector.tensor_scalar(out=dg[:], in0=P[:, D:D + 1],
                                                scalar1=1.0, scalar2=None,
                                                op0=OP.add)
                        ZT = tpool.tile([128, 128], F32, tag="t")
                        nc.tensor.transpose(ZT[:], Z[:], ident[:])
                        ZTs = ztpool.tile([128, 128], F32, tag="zt")
                        nc.vector.tensor_copy(out=ZTs[:], in_=ZT[:])
                        nc.tensor.matmul(q_ps[:, j * 128:(j + 1) * 128],
                                         lhsT=ZTs[:], rhs=W_sb[:],
                                         start=True, stop=True)
                        nc.vector.tensor_scalar(
                            out=bt[:, j * 128:(j + 1) * 128], in0=brep_sb[:],
                            scalar1=dg[:, 0:1], scalar2=None, op0=OP.mult)
                    # epilogue: q2 = q + bt ; h = selu(q2)
                    q2 = epool.tile([128, GRP * 128], F32, tag="q2")
                    nc.vector.tensor_tensor(out=q2[:, :w], in0=q_ps[:, :w],
                                            in1=bt[:, :w], op=OP.add)
                    m = epool.tile([128, GRP * 128], F32, tag="m")
                    nc.vector.tensor_scalar(out=m[:, :w], in0=q2[:, :w],
                                            scalar1=0.0, scalar2=None,
                                            op0=OP.min)
                    ex = epool.tile([128, GRP * 128], F32, tag="ex")
                    nc.scalar.activation(out=ex[:, :w], in_=m[:, :w],
                                         func=ACT.Exp)
                    nc.vector.tensor_scalar(out=ex[:, :w], in0=ex[:, :w],
                                            scalar1=SA, scalar2=SA,
                                            op0=OP.mult, op1=OP.subtract)
                    nc.vector.tensor_scalar(out=q2[:, :w], in0=q2[:, :w],
                                            scalar1=0.0, scalar2=SELU_SCALE,
                                            op0=OP.max, op1=OP.mult)
                    h_write(g0, gb, ex, q2)
                    if dram_out is not None:
                        dram_out(g0, gb)

            # ---------------- layer 1 ----------------
            def feats_own1(b):
                xo = xopool.tile([128, D], F32, tag="xo")
                nc.sync.dma_start(out=xo[:],
                                  in_=xown[b * 128:(b + 1) * 128, :])
                return xo[:]

            def h1_write(g0, gb, ex, q2):
                w = gb * 128
                nc.vector.tensor_tensor(
                    out=h1_sb[:, g0:g0 + gb, 0:D],
                    in0=ex[:, :w].rearrange("p (g d) -> p g d", g=gb),
                    in1=q2[:, :w].rearrange("p (g d) -> p g d", g=gb),
                    op=OP.add)

            h1own_t = h1own[:, :].rearrange("(b p) r -> p b r", p=128)

            def h1_dram(g0, gb):
                nc.sync.dma_start(out=h1own_t[:, g0:g0 + gb, :],
                                  in_=h1_sb[:, g0:g0 + gb, :])

            layer(xfull, feats_own1, w1_sb, b1r_sb, h1_write, h1_dram)

            nc.gpsimd.collective_compute(
                "AllGather", mybir.AluOpType.bypass,
                replica_groups=[list(range(C))],
                ins=[h1own[:, :]], outs=[h1full[:, :]])

            # ---------------- layer 2 ----------------
            def feats_own2(b):
                return h1_sb[:, b, 0:D]

            def h2_write(g0, gb, ex, q2):
                w = gb * 128
                nc.vector.tensor_tensor(
                    out=h2_sb[:, g0:g0 + gb, :],
                    in0=ex[:, :w].rearrange("p (g d) -> p g d", g=gb),
                    in1=q2[:, :w].rearrange("p (g d) -> p g d", g=gb),
                    op=OP.add)

            layer(h1full, feats_own2, w2_sb, b2r_sb, h2_write, None)

            # ---------------- assignment head ----------------
            for b in range(B):
                ZT = tpool.tile([128, 128], F32, tag="t")
                nc.tensor.transpose(ZT[:], h2_sb[:, b, :], ident[:])
                ZTs = ztpool.tile([128, 128], F32, tag="zt")
                nc.vector.tensor_copy(out=ZTs[:], in_=ZT[:])
                za = qpool.tile([128, GRP * 128], F32, tag="q")
                nc.tensor.matmul(za[:, 0:KCL], lhsT=ZTs[:], rhs=wa_sb[:, :],
                                 start=True, stop=True)
                zs = l3pool.tile([128, KCL], F32, tag="zs")
                nc.vector.tensor_tensor(out=zs[:], in0=za[:, 0:KCL],
                                        in1=bar_sb[:], op=OP.add)
                mx = l3pool.tile([128, 1], F32, tag="mx")
                nc.vector.tensor_reduce(out=mx[:], in_=zs[:],
                                        axis=mybir.AxisListType.X, op=OP.max)
                nc.vector.tensor_scalar(out=zs[:], in0=zs[:],
                                        scalar1=mx[:, 0:1], scalar2=None,
                                        op0=OP.subtract)
                es = l3pool.tile([128, KCL], F32, tag="es")
                nc.scalar.activation(out=es[:], in_=zs[:], func=ACT.Exp)
                sm = l3pool.tile([128, 1], F32, tag="sm")
                nc.vector.tensor_reduce(out=sm[:], in_=es[:],
                                        axis=mybir.AxisListType.X, op=OP.add)
                rc = l3pool.tile([128, 1], F32, tag="rc")
                nc.vector.reciprocal(out=rc[:], in_=sm[:])
                oo = l3pool.tile([128, KCL], F32, tag="oo")
                nc.vector.tensor_scalar(out=oo[:], in0=es[:],
                                        scalar1=rc[:, 0:1], scalar2=None,
                                        op0=OP.mult)
                nc.sync.dma_start(out=outp[b * 128:(b + 1) * 128, :],
                                  in_=oo[:])

    nc.compile()
    return nc


# ----------------------------------------------------------------------------
# public entry point
# ----------------------------------------------------------------------------

def make_in_maps(meta, SRC, REL, WGT, xfull, xown, W1, b1, W2, b2, Wa, ba):
    iota = np.tile(np.arange(D, dtype=np.float32), (128, 1))
    b1r = np.tile(np.asarray(b1, np.float32), (128, 1))
    b2r = np.tile(np.asarray(b2, np.float32), (128, 1))
    bar = np.tile(np.asarray(ba, np.float32), (128, 1))
    in_maps = []
    for c in range(C):
        in_maps.append({
            "xfull": xfull, "xown": xown[c],
            "srci": SRC[c], "reli": REL[c], "wgti": WGT[c],
            "w1": np.asarray(W1, np.float32), "w2": np.asarray(W2, np.float32),
            "wa": np.asarray(Wa, np.float32),
            "b1r": b1r, "b2r": b2r, "bar": bar, "iot": iota,
        })
    return in_maps


def get_compiled(x, edge_index, edge_weight):
    meta, SRC, REL, WGT, xfull, xown = preprocess(x, edge_index, edge_weight)
    key = (meta["B"], meta["TPB"], meta["NPC"], meta["Npad"])
    if key not in _CACHE:
        _CACHE[key] = build_program(*key)
    return _CACHE[key], meta, SRC, REL, WGT, xfull, xown


def kernel(x, edge_index, edge_weight, W1, b1, skip1, W2, b2, skip2, Wa, ba):
    x = np.asarray(x)
    edge_index = np.asarray(edge_index)
    edge_weight = np.asarray(edge_weight)
    if (x.shape[0] % C != 0 or x.shape[1] != D
            or not np.allclose(np.asarray(skip1), 1.0)
            or not np.allclose(np.asarray(skip2), 1.0)):
        return _numpy_reference(x, edge_index, edge_weight, W1, b1, skip1,
                                W2, b2, skip2, Wa, ba)

    from concourse.bass_utils import run_bass_kernel_spmd

    nc, meta, SRC, REL, WGT, xfull, xown = get_compiled(
        x, edge_index, edge_weight)
    in_maps = make_in_maps(meta, SRC, REL, WGT, xfull, xown,
                           W1, b1, W2, b2, Wa, ba)
    res = run_bass_kernel_spmd(nc, in_maps, core_ids=list(range(C)))
    npc_real = meta["npc_real"]
    out = np.concatenate(
        [res.results[c]["assign"][:npc_real] for c in range(C)], axis=0)
    return np.ascontiguousarray(out.astype(np.float32))


# ----------------------------------------------------------------------------
# numpy fallback (also used to validate the device path in tests)
# ----------------------------------------------------------------------------

def _numpy_reference(x, edge_index, edge_weight, W1, b1, skip1, W2, b2,
                     skip2, Wa, ba):
    x = np.asarray(x, np.float64)
    dst, src = edge_index[0], edge_index[1]
    w = np.asarray(edge_weight, np.float64)
    N = x.shape[0]

    def spmm(feats):
        out = np.zeros_like(feats)
        np.add.at(out, dst, feats[src] * w[:, None])
        return out

    def selu(v):
        return SELU_SCALE * np.where(v > 0, v, SELU_ALPHA * (np.exp(v) - 1))

    def gcn(feats, W, b, skip):
        t = feats @ np.asarray(W, np.float64) + np.asarray(b, np.float64)
        return selu(np.asarray(skip, np.float64) * t + spmm(t))

    h = gcn(x, W1, b1, skip1)
    h = gcn(h, W2, b2, skip2)
    z = h @ np.asarray(Wa, np.float64) + np.asarray(ba, np.float64)
    z = z - z.max(axis=1, keepdims=True)
    ez = np.exp(z)
    return (ez / ez.sum(axis=1, keepdims=True)).astype(np.float32)


# revision 15
# speedup vs baseline: 1.1021x; 1.1021x over previous
"""Distributed GCN (DMoN front-end) kernel for 8 Trainium2 NeuronCores.

Strategy (matches the sharding hint):
  - Nodes are partitioned contiguously across the 8 cores; each core owns its
    nodes' incident (incoming) edges, grouped by destination block of 128.
  - spmm is computed as a sequence of one-hot "segment matmuls": for each tile
    of 128 edges (sorted by destination), build S[e, d] = w_e * (rel_dst[e]==d)
    on the vector engine and accumulate P += S^T @ gathered_rows on the PE,
    where gathered_rows come from a dma_gather of source-node feature rows
    (one bulk gather per destination block and per index half, since
    dma_gather indices are int16).
  - The linear transform is applied AFTER aggregation (linearity):
        selu(skip*(xW+b) + spmm(xW+b)) == selu((x_own + spmm_raw(x))W +
        (deg_w + skip)*b)          (skip == 1 in this model)
    so the gather tables are the raw features (x, then h1) — no transformed
    table is ever materialized.
  - deg_w (sum of incident edge weights per node) is produced by a second
    PSUM accumulation chain (S^T @ ones) in layer 1 and reused in layer 2.
  - Between the two GCN layers, h1 is AllGathered across the 8 cores so every
    core can gather any source row of h1.
"""

import math

import numpy as np

C = 8            # cores
D = 128          # feature dim
KCL = 16         # clusters
GRP = 4          # dst blocks per PSUM epilogue group
HALF = 32768     # int16 index range per gather table slice
SELU_ALPHA = 1.6732632423543772
SELU_SCALE = 1.0507009873554805

_CACHE = {}


# ----------------------------------------------------------------------------
# host-side preprocessing (pure index manipulation + layout)
# ----------------------------------------------------------------------------

def preprocess(x, edge_index, edge_weight):
    N = x.shape[0]
    E = edge_index.shape[1]
    npc_real = N // C
    assert N % C == 0
    B = math.ceil(npc_real / 128)
    NPC = 128 * B
    Npad = C * NPC
    nhalf = 2 if Npad > HALF else 1
    assert Npad <= 2 * HALF

    dst = edge_index[0].astype(np.int64)
    src = edge_index[1].astype(np.int64)
    core = dst // npc_real
    dst_loc = dst - core * npc_real
    blk = dst_loc // 128
    rel = (dst_loc % 128).astype(np.float32)

    src_pad = (src // npc_real) * NPC + (src % npc_real)
    half = (src_pad >= HALF).astype(np.int64)

    gkey = (core * B + blk) * 2 + half
    counts = np.bincount(gkey, minlength=C * B * 2)
    c2 = counts.reshape(C * B, 2)
    TPB_LO = max(1, int(math.ceil(c2[:, 0].max() / 128)))
    TPB_HI = int(math.ceil(c2[:, 1].max() / 128)) if nhalf == 2 else 0
    TT = TPB_LO + TPB_HI
    T = B * TT

    order = np.argsort(gkey, kind="stable")
    starts = np.zeros(C * B * 2 + 1, np.int64)
    starts[1:] = np.cumsum(counts)
    pos = np.arange(E, dtype=np.int64) - starts[gkey[order]]

    core_s = core[order]
    blk_s = blk[order]
    half_s = half[order]
    col = blk_s * TT + half_s * TPB_LO + pos // 128
    row = pos % 128

    REL = np.zeros((C, 128, T), np.float32)
    WGT = np.zeros((C, 128, T), np.float32)
    REL[core_s, row, col] = rel[order]
    WGT[core_s, row, col] = np.asarray(edge_weight, np.float32)[order]

    # int16 gather indices in dma_gather's wrapped layout:
    # within one call, edge slot q maps to idx16[q % 16, c0 + q // 16]; the
    # 16-row pattern is replicated to all 128 partitions (one copy per Q7
    # core).
    IDX = np.zeros((C, 16, T * 8), np.int16)
    relsrc = (src_pad[order] - half_s * HALF).astype(np.int16)
    q = pos
    c0 = (blk_s * TT + half_s * TPB_LO) * 8
    IDX[core_s, q % 16, c0 + q // 16] = relsrc
    IDX = np.tile(IDX, (1, 8, 1))               # [C, 128, T*8]

    # padded gather table for layer 1
    xt = np.zeros((C, NPC, D), np.float32)
    xt[:, :npc_real, :] = np.asarray(x, np.float32).reshape(C, npc_real, D)
    xfull = np.ascontiguousarray(xt.reshape(Npad, D))

    meta = dict(N=N, E=E, npc_real=npc_real, B=B, NPC=NPC, Npad=Npad,
                TPB_LO=TPB_LO, TPB_HI=TPB_HI, TT=TT, T=T)
    return meta, IDX, REL, WGT, xfull, xt


# ----------------------------------------------------------------------------
# bass program
# ----------------------------------------------------------------------------

def build_program(B, TPB_LO, TPB_HI, NPC, Npad, no_collective=False):
    import concourse.bass as bass
    import concourse.bacc as bacc
    import concourse.mybir as mybir
    from concourse.tile import TileContext
    from concourse.masks import make_identity

    F32 = mybir.dt.float32
    I16 = mybir.dt.int16
    OP = mybir.AluOpType
    ACT = mybir.ActivationFunctionType
    TT = TPB_LO + TPB_HI
    T = B * TT
    SA = SELU_SCALE * SELU_ALPHA
    halves = [(0, min(Npad, HALF), 0, TPB_LO)]
    if TPB_HI:
        halves.append((HALF, Npad - HALF, TPB_LO, TPB_HI))

    nc = bacc.Bacc("TRN2", target_bir_lowering=False, debug=False,
                   num_devices=C)

    xfull = nc.dram_tensor("xfull", [Npad, D], F32, kind="ExternalInput")
    xown = nc.dram_tensor("xown", [NPC, D], F32, kind="ExternalInput")
    idx16 = nc.dram_tensor("idx16", [128, T * 8], I16, kind="ExternalInput")
    reli = nc.dram_tensor("reli", [128, T], F32, kind="ExternalInput")
    wgti = nc.dram_tensor("wgti", [128, T], F32, kind="ExternalInput")
    w1 = nc.dram_tensor("w1", [D, D], F32, kind="ExternalInput")
    w2 = nc.dram_tensor("w2", [D, D], F32, kind="ExternalInput")
    wa = nc.dram_tensor("wa", [D, KCL], F32, kind="ExternalInput")
    b1r = nc.dram_tensor("b1r", [128, D], F32, kind="ExternalInput")
    b2r = nc.dram_tensor("b2r", [128, D], F32, kind="ExternalInput")
    bar = nc.dram_tensor("bar", [128, KCL], F32, kind="ExternalInput")
    iot = nc.dram_tensor("iot", [128, D], F32, kind="ExternalInput")

    h1own = nc.dram_tensor("h1own", [NPC, D], F32)
    h1full = nc.dram_tensor("h1full", [Npad, D], F32, addr_space="Shared")
    outp = nc.dram_tensor("assign", [NPC, KCL], F32, kind="ExternalOutput")

    with TileContext(nc) as tc:
        with tc.tile_pool(name="const", bufs=1) as const, \
             tc.tile_pool(name="g", bufs=2) as gpool, \
             tc.tile_pool(name="s", bufs=8) as spool, \
             tc.tile_pool(name="z", bufs=4) as zpool, \
             tc.tile_pool(name="zt", bufs=4) as ztpool, \
             tc.tile_pool(name="bt", bufs=2) as btpool, \
             tc.tile_pool(name="ep", bufs=2) as epool, \
             tc.tile_pool(name="xo", bufs=4) as xopool, \
             tc.tile_pool(name="l3", bufs=3) as l3pool, \
             tc.tile_pool(name="pp", bufs=2, space="PSUM") as ppool, \
             tc.tile_pool(name="pd", bufs=2, space="PSUM") as pdpool, \
             tc.tile_pool(name="qp", bufs=2, space="PSUM") as qpool, \
             tc.tile_pool(name="tp", bufs=2, space="PSUM") as tpool:

            # ---- persistent state ----
            idx_sb = const.tile([128, T * 8], I16)
            nc.sync.dma_start(out=idx_sb[:], in_=idx16[:, :])
            rel_sb = const.tile([128, T], F32)
            nc.sync.dma_start(out=rel_sb[:], in_=reli[:, :])
            wgt_sb = const.tile([128, T], F32)
            nc.sync.dma_start(out=wgt_sb[:], in_=wgti[:, :])
            iota_sb = const.tile([128, D], F32)
            nc.sync.dma_start(out=iota_sb[:], in_=iot[:, :])
            w1_sb = const.tile([128, D], F32)
            nc.sync.dma_start(out=w1_sb[:], in_=w1[:, :])
            w2_sb = const.tile([128, D], F32)
            nc.sync.dma_start(out=w2_sb[:], in_=w2[:, :])
            wa_sb = const.tile([128, KCL], F32)
            nc.sync.dma_start(out=wa_sb[:], in_=wa[:, :])
            b1r_sb = const.tile([128, D], F32)
            nc.sync.dma_start(out=b1r_sb[:], in_=b1r[:, :])
            b2r_sb = const.tile([128, D], F32)
            nc.sync.dma_start(out=b2r_sb[:], in_=b2r[:, :])
            bar_sb = const.tile([128, KCL], F32)
            nc.sync.dma_start(out=bar_sb[:], in_=bar[:, :])
            ident = const.tile([128, 128], F32)
            make_identity(nc, ident[:])
            ones_sb = const.tile([128, 1], F32)
            nc.vector.memset(ones_sb[:], 1.0)
            degw1_sb = const.tile([128, B], F32)

            h1_sb = const.tile([128, B, D], F32)
            h2_sb = const.tile([128, B, D], F32)

            def layer(first, table, feats_own, W_sb, brep_sb, h_write,
                      dram_out):
                for g0 in range(0, B, GRP):
                    gb = min(GRP, B - g0)
                    w = gb * 128
                    q_ps = qpool.tile([128, GRP * 128], F32, tag="q")
                    bt = btpool.tile([128, GRP * 128], F32, tag="bt")
                    for j in range(gb):
                        b = g0 + j
                        gt = gpool.tile([128, TT, D], F32, tag="g")
                        # dma_gather calls above ~1024 indices hang the HW;
                        # split into <=8-tile (1024-idx) sub-calls.
                        SUBT = 8
                        for (base, size, toff, tcnt) in halves:
                            for s0 in range(0, tcnt, SUBT):
                                sc = min(SUBT, tcnt - s0)
                                o = toff + s0
                                nc.gpsimd.dma_gather(
                                    out_ap=gt[:, o:o + sc, :],
                                    in_ap=table[base:base + size, :],
                                    idxs_ap=idx_sb[:, (b * TT + o) * 8:
                                                   (b * TT + o + sc) * 8],
                                    num_idxs=sc * 128,
                                    num_idxs_reg=sc * 128,
                                    elem_size=D)
                        P = ppool.tile([128, 128], F32, tag="p")
                        Pd = None
                        if first:
                            Pd = pdpool.tile([128, 1], F32, tag="pd")
                        for t in range(TT):
                            tcol = b * TT + t
                            St = spool.tile([128, 128], F32, tag="s")
                            nc.vector.tensor_scalar(
                                out=St[:], in0=iota_sb[:],
                                scalar1=rel_sb[:, tcol:tcol + 1],
                                scalar2=wgt_sb[:, tcol:tcol + 1],
                                op0=OP.is_equal, op1=OP.mult)
                            nc.tensor.matmul(
                                P[:, 0:D], lhsT=St[:], rhs=gt[:, t, :],
                                start=(t == 0), stop=(t == TT - 1),
                                skip_group_check=True)
                            if first:
                                nc.tensor.matmul(
                                    Pd[:, 0:1], lhsT=St[:],
                                    rhs=ones_sb[:, 0:1],
                                    start=(t == 0), stop=(t == TT - 1),
                                    skip_group_check=True)
                        fo = feats_own(b)
                        Z = zpool.tile([128, D], F32, tag="z")
                        nc.vector.tensor_tensor(out=Z[:], in0=P[:, 0:D],
                                                in1=fo, op=OP.add)
                        if first:
                            nc.vector.tensor_scalar(
                                out=degw1_sb[:, b:b + 1], in0=Pd[:, 0:1],
                                scalar1=1.0, scalar2=None, op0=OP.add)
                        ZT = tpool.tile([128, 128], F32, tag="t")
                        nc.tensor.transpose(ZT[:], Z[:], ident[:])
                        ZTs = ztpool.tile([128, 128], F32, tag="zt")
                        nc.vector.tensor_copy(out=ZTs[:], in_=ZT[:])
                        nc.tensor.matmul(q_ps[:, j * 128:(j + 1) * 128],
                                         lhsT=ZTs[:], rhs=W_sb[:],
                                         start=True, stop=True)
                        nc.vector.tensor_scalar(
                            out=bt[:, j * 128:(j + 1) * 128], in0=brep_sb[:],
                            scalar1=degw1_sb[:, b:b + 1], scalar2=None,
                            op0=OP.mult)
                    # epilogue: q2 = q + bt ; h = selu(q2)
                    q2 = epool.tile([128, GRP * 128], F32, tag="q2")
                    nc.vector.tensor_tensor(out=q2[:, :w], in0=q_ps[:, :w],
                                            in1=bt[:, :w], op=OP.add)
                    m = epool.tile([128, GRP * 128], F32, tag="m")
                    nc.vector.tensor_scalar(out=m[:, :w], in0=q2[:, :w],
                                            scalar1=0.0, scalar2=None,
                                            op0=OP.min)
                    ex = epool.tile([128, GRP * 128], F32, tag="ex")
                    nc.scalar.activation(out=ex[:, :w], in_=m[:, :w],
                                         func=ACT.Exp)
                    nc.vector.tensor_scalar(out=ex[:, :w], in0=ex[:, :w],
                                            scalar1=SA, scalar2=SA,
                                            op0=OP.mult, op1=OP.subtract)
                    nc.vector.tensor_scalar(out=q2[:, :w], in0=q2[:, :w],
                                            scalar1=0.0, scalar2=SELU_SCALE,
                                            op0=OP.max, op1=OP.mult)
                    h_write(g0, gb, ex, q2)
                    if dram_out is not None:
                        dram_out(g0, gb)

            # ---------------- layer 1 ----------------
            def feats_own1(b):
                xo = xopool.tile([128, D], F32, tag="xo")
                nc.sync.dma_start(out=xo[:],
                                  in_=xown[b * 128:(b + 1) * 128, :])
                return xo[:]

            def h1_write(g0, gb, ex, q2):
                w = gb * 128
                nc.vector.tensor_tensor(
                    out=h1_sb[:, g0:g0 + gb, :],
                    in0=ex[:, :w].rearrange("p (g d) -> p g d", g=gb),
                    in1=q2[:, :w].rearrange("p (g d) -> p g d", g=gb),
                    op=OP.add)

            h1own_t = h1own[:, :].rearrange("(b p) r -> p b r", p=128)

            def h1_dram(g0, gb):
                nc.sync.dma_start(out=h1own_t[:, g0:g0 + gb, :],
                                  in_=h1_sb[:, g0:g0 + gb, :])

            layer(True, xfull, feats_own1, w1_sb, b1r_sb, h1_write, h1_dram)

            if no_collective:
                # timeline-sim stand-in keeping the dataflow dependency
                nc.sync.dma_start(out=h1full[0:NPC, :], in_=h1own[:, :])
            else:
                nc.gpsimd.collective_compute(
                    "AllGather", mybir.AluOpType.bypass,
                    replica_groups=[list(range(C))],
                    ins=[h1own[:, :]], outs=[h1full[:, :]])

            # ---------------- layer 2 ----------------
            def feats_own2(b):
                return h1_sb[:, b, :]

            def h2_write(g0, gb, ex, q2):
                w = gb * 128
                nc.vector.tensor_tensor(
                    out=h2_sb[:, g0:g0 + gb, :],
                    in0=ex[:, :w].rearrange("p (g d) -> p g d", g=gb),
                    in1=q2[:, :w].rearrange("p (g d) -> p g d", g=gb),
                    op=OP.add)

            layer(False, h1full, feats_own2, w2_sb, b2r_sb, h2_write, None)

            # ---------------- assignment head ----------------
            for b in range(B):
                ZT = tpool.tile([128, 128], F32, tag="t")
                nc.tensor.transpose(ZT[:], h2_sb[:, b, :], ident[:])
                ZTs = ztpool.tile([128, 128], F32, tag="zt")
                nc.vector.tensor_copy(out=ZTs[:], in_=ZT[:])
                za = qpool.tile([128, GRP * 128], F32, tag="q")
                nc.tensor.matmul(za[:, 0:KCL], lhsT=ZTs[:], rhs=wa_sb[:, :],
                                 start=True, stop=True)
                zs = l3pool.tile([128, KCL], F32, tag="zs")
                nc.vector.tensor_tensor(out=zs[:], in0=za[:, 0:KCL],
                                        in1=bar_sb[:], op=OP.add)
                mx = l3pool.tile([128, 1], F32, tag="mx")
                nc.vector.tensor_reduce(out=mx[:], in_=zs[:],
                                        axis=mybir.AxisListType.X, op=OP.max)
                nc.vector.tensor_scalar(out=zs[:], in0=zs[:],
                                        scalar1=mx[:, 0:1], scalar2=None,
                                        op0=OP.subtract)
                es = l3pool.tile([128, KCL], F32, tag="es")
                nc.scalar.activation(out=es[:], in_=zs[:], func=ACT.Exp)
                sm = l3pool.tile([128, 1], F32, tag="sm")
                nc.vector.tensor_reduce(out=sm[:], in_=es[:],
                                        axis=mybir.AxisListType.X, op=OP.add)
                rc = l3pool.tile([128, 1], F32, tag="rc")
                nc.vector.reciprocal(out=rc[:], in_=sm[:])
                oo = l3pool.tile([128, KCL], F32, tag="oo")
                nc.vector.tensor_scalar(out=oo[:], in0=es[:],
                                        scalar1=rc[:, 0:1], scalar2=None,
                                        op0=OP.mult)
                nc.sync.dma_start(out=outp[b * 128:(b + 1) * 128, :],
                                  in_=oo[:])

    nc.compile()
    return nc


# ----------------------------------------------------------------------------
# public entry point
# ----------------------------------------------------------------------------

def make_in_maps(meta, IDX, REL, WGT, xfull, xown, W1, b1, W2, b2, Wa, ba):
    iota = np.tile(np.arange(D, dtype=np.float32), (128, 1))
    b1r = np.tile(np.asarray(b1, np.float32), (128, 1))
    b2r = np.tile(np.asarray(b2, np.float32), (128, 1))
    bar = np.tile(np.asarray(ba, np.float32), (128, 1))
    in_maps = []
    for c in range(C):
        in_maps.append({
            "xfull": xfull, "xown": xown[c],
            "idx16": IDX[c], "reli": REL[c], "wgti": WGT[c],
            "w1": np.asarray(W1, np.float32), "w2": np.asarray(W2, np.float32),
            "wa": np.asarray(Wa, np.float32),
            "b1r": b1r, "b2r": b2r, "bar": bar, "iot": iota,
        })
    return in_maps


def get_compiled(x, edge_index, edge_weight):
    meta, IDX, REL, WGT, xfull, xown = preprocess(x, edge_index, edge_weight)
    key = (meta["B"], meta["TPB_LO"], meta["TPB_HI"], meta["NPC"],
           meta["Npad"])
    if key not in _CACHE:
        _CACHE[key] = build_program(*key)
    return _CACHE[key], meta, IDX, REL, WGT, xfull, xown


def kernel(x, edge_index, edge_weight, W1, b1, skip1, W2, b2, skip2, Wa, ba):
    x = np.asarray(x)
    edge_index = np.asarray(edge_index)
    edge_weight = np.asarray(edge_weight)
    if (x.shape[0] % C != 0 or x.shape[1] != D
            or x.shape[0] > 2 * HALF - 128 * C   # Npad must fit 2 halves
            or not np.allclose(np.asarray(skip1), 1.0)
            or not np.allclose(np.asarray(skip2), 1.0)):
        return _numpy_reference(x, edge_index, edge_weight, W1, b1, skip1,
                                W2, b2, skip2, Wa, ba)

    from concourse.bass_utils import run_bass_kernel_spmd

    nc, meta, IDX, REL, WGT, xfull, xown = get_compiled(
        x, edge_index, edge_weight)
    in_maps = make_in_maps(meta, IDX, REL, WGT, xfull, xown,
                           W1, b1, W2, b2, Wa, ba)
    res = run_bass_kernel_spmd(nc, in_maps, core_ids=list(range(C)))
    npc_real = meta["npc_real"]
    out = np.concatenate(
        [res.results[c]["assign"][:npc_real] for c in range(C)], axis=0)
    return np.ascontiguousarray(out.astype(np.float32))


# ----------------------------------------------------------------------------
# numpy fallback (also used to validate the device path in tests)
# ----------------------------------------------------------------------------

def _numpy_reference(x, edge_index, edge_weight, W1, b1, skip1, W2, b2,
                     skip2, Wa, ba):
    x = np.asarray(x, np.float64)
    dst, src = edge_index[0], edge_index[1]
    w = np.asarray(edge_weight, np.float64)

    def spmm(feats):
        out = np.zeros_like(feats)
        np.add.at(out, dst, feats[src] * w[:, None])
        return out

    def selu(v):
        return SELU_SCALE * np.where(v > 0, v, SELU_ALPHA * (np.exp(v) - 1))

    def gcn(feats, W, b, skip):
        t = feats @ np.asarray(W, np.float64) + np.asarray(b, np.float64)
        return selu(np.asarray(skip, np.float64) * t + spmm(t))

    h = gcn(x, W1, b1, skip1)
    h = gcn(h, W2, b2, skip2)
    z = h @ np.asarray(Wa, np.float64) + np.asarray(ba, np.float64)
    z = z - z.max(axis=1, keepdims=True)
    ez = np.exp(z)
    return (ez / ez.sum(axis=1, keepdims=True)).astype(np.float32)


# revision 18
# speedup vs baseline: 1.3015x; 1.1810x over previous
"""Distributed GCN (DMoN front-end) kernel for 8 Trainium2 NeuronCores.

Strategy (matches the sharding hint):
  - Nodes are partitioned contiguously across the 8 cores; each core owns its
    nodes' incident (incoming) edges, grouped by destination block of 128.
  - spmm is computed as a sequence of one-hot "segment matmuls": for each tile
    of 128 edges (sorted by destination), build S[e, d] = w_e * (rel_dst[e]==d)
    on the vector engine and accumulate P += S^T @ gathered_rows on the PE,
    where gathered_rows come from a dma_gather of source-node feature rows
    (one bulk gather per destination block and per index half, since
    dma_gather indices are int16).
  - The linear transform is applied AFTER aggregation (linearity):
        selu(skip*(xW+b) + spmm(xW+b)) == selu((x_own + spmm_raw(x))W +
        (deg_w + skip)*b)          (skip == 1 in this model)
    so the gather tables are the raw features (x, then h1) — no transformed
    table is ever materialized.
  - deg_w (sum of incident edge weights per node) is produced by a second
    PSUM accumulation chain (S^T @ ones) in layer 1 and reused in layer 2.
  - Between the two GCN layers, h1 is AllGathered across the 8 cores so every
    core can gather any source row of h1.
"""

import math

import numpy as np

C = 8            # cores
D = 128          # feature dim
KCL = 16         # clusters
GRP = 4          # dst blocks per PSUM epilogue group
HALF = 32768     # int16 index range per gather table slice
SELU_ALPHA = 1.6732632423543772
SELU_SCALE = 1.0507009873554805

_CACHE = {}


# ----------------------------------------------------------------------------
# host-side preprocessing (pure index manipulation + layout)
# ----------------------------------------------------------------------------

def preprocess(x, edge_index, edge_weight):
    N = x.shape[0]
    E = edge_index.shape[1]
    npc_real = N // C
    assert N % C == 0
    B = math.ceil(npc_real / 128)
    NPC = 128 * B
    Npad = C * NPC
    nhalf = 2 if Npad > HALF else 1
    assert Npad <= 2 * HALF

    dst = edge_index[0].astype(np.int64)
    src = edge_index[1].astype(np.int64)
    core = dst // npc_real
    dst_loc = dst - core * npc_real
    blk = dst_loc // 128
    rel = (dst_loc % 128).astype(np.float32)

    src_pad = (src // npc_real) * NPC + (src % npc_real)
    half = (src_pad >= HALF).astype(np.int64)

    gkey = (core * B + blk) * 2 + half
    counts = np.bincount(gkey, minlength=C * B * 2)
    c2 = counts.reshape(C * B, 2)
    TPB_LO = max(1, int(math.ceil(c2[:, 0].max() / 128)))
    TPB_HI = int(math.ceil(c2[:, 1].max() / 128)) if nhalf == 2 else 0
    TT = TPB_LO + TPB_HI
    T = B * TT

    order = np.argsort(gkey, kind="stable")
    starts = np.zeros(C * B * 2 + 1, np.int64)
    starts[1:] = np.cumsum(counts)
    pos = np.arange(E, dtype=np.int64) - starts[gkey[order]]

    core_s = core[order]
    blk_s = blk[order]
    half_s = half[order]
    col = blk_s * TT + half_s * TPB_LO + pos // 128
    row = pos % 128

    REL = np.zeros((C, 128, T), np.float32)
    WGT = np.zeros((C, 128, T), np.float32)
    REL[core_s, row, col] = rel[order]
    WGT[core_s, row, col] = np.asarray(edge_weight, np.float32)[order]

    # int16 gather indices in dma_gather's wrapped layout:
    # within one call, edge slot q maps to idx16[q % 16, c0 + q // 16]; the
    # 16-row pattern is replicated to all 128 partitions (one copy per Q7
    # core).
    IDX = np.zeros((C, 16, T * 8), np.int16)
    relsrc = (src_pad[order] - half_s * HALF).astype(np.int16)
    q = pos
    c0 = (blk_s * TT + half_s * TPB_LO) * 8
    IDX[core_s, q % 16, c0 + q // 16] = relsrc
    IDX = np.tile(IDX, (1, 8, 1))               # [C, 128, T*8]

    # padded gather table for layer 1
    xt = np.zeros((C, NPC, D), np.float32)
    xt[:, :npc_real, :] = np.asarray(x, np.float32).reshape(C, npc_real, D)
    xfull = np.ascontiguousarray(xt.reshape(Npad, D))

    meta = dict(N=N, E=E, npc_real=npc_real, B=B, NPC=NPC, Npad=Npad,
                TPB_LO=TPB_LO, TPB_HI=TPB_HI, TT=TT, T=T)
    return meta, IDX, REL, WGT, xfull, xt


# ----------------------------------------------------------------------------
# bass program
# ----------------------------------------------------------------------------

def build_program(B, TPB_LO, TPB_HI, NPC, Npad, no_collective=False):
    import concourse.bass as bass
    import concourse.bacc as bacc
    import concourse.mybir as mybir
    from concourse.tile import TileContext
    from concourse.masks import make_identity

    F32 = mybir.dt.float32
    I16 = mybir.dt.int16
    OP = mybir.AluOpType
    ACT = mybir.ActivationFunctionType
    TT = TPB_LO + TPB_HI
    T = B * TT
    SA = SELU_SCALE * SELU_ALPHA
    halves = [(0, min(Npad, HALF), 0, TPB_LO)]
    if TPB_HI:
        halves.append((HALF, Npad - HALF, TPB_LO, TPB_HI))

    nc = bacc.Bacc("TRN2", target_bir_lowering=False, debug=False,
                   num_devices=C, num_swdge_queues=2)

    xfull = nc.dram_tensor("xfull", [Npad, D], F32, kind="ExternalInput")
    xown = nc.dram_tensor("xown", [NPC, D], F32, kind="ExternalInput")
    idx16 = nc.dram_tensor("idx16", [128, T * 8], I16, kind="ExternalInput")
    reli = nc.dram_tensor("reli", [128, T], F32, kind="ExternalInput")
    wgti = nc.dram_tensor("wgti", [128, T], F32, kind="ExternalInput")
    w1 = nc.dram_tensor("w1", [D, D], F32, kind="ExternalInput")
    w2 = nc.dram_tensor("w2", [D, D], F32, kind="ExternalInput")
    wa = nc.dram_tensor("wa", [D, KCL], F32, kind="ExternalInput")
    b1r = nc.dram_tensor("b1r", [128, D], F32, kind="ExternalInput")
    b2r = nc.dram_tensor("b2r", [128, D], F32, kind="ExternalInput")
    bar = nc.dram_tensor("bar", [128, KCL], F32, kind="ExternalInput")
    iot = nc.dram_tensor("iot", [128, D], F32, kind="ExternalInput")

    h1own = nc.dram_tensor("h1own", [NPC, D], F32)
    h1full = nc.dram_tensor("h1full", [Npad, D], F32, addr_space="Shared")
    outp = nc.dram_tensor("assign", [NPC, KCL], F32, kind="ExternalOutput")

    with TileContext(nc) as tc:
        with tc.tile_pool(name="const", bufs=1) as const, \
             tc.tile_pool(name="g", bufs=2) as gpool, \
             tc.tile_pool(name="s", bufs=8) as spool, \
             tc.tile_pool(name="z", bufs=4) as zpool, \
             tc.tile_pool(name="zt", bufs=4) as ztpool, \
             tc.tile_pool(name="bt", bufs=2) as btpool, \
             tc.tile_pool(name="ep", bufs=2) as epool, \
             tc.tile_pool(name="xo", bufs=4) as xopool, \
             tc.tile_pool(name="l3", bufs=3) as l3pool, \
             tc.tile_pool(name="pp", bufs=2, space="PSUM") as ppool, \
             tc.tile_pool(name="pd", bufs=2, space="PSUM") as pdpool, \
             tc.tile_pool(name="qp", bufs=2, space="PSUM") as qpool, \
             tc.tile_pool(name="tp", bufs=2, space="PSUM") as tpool:

            # ---- persistent state ----
            idx_sb = const.tile([128, T * 8], I16)
            nc.sync.dma_start(out=idx_sb[:], in_=idx16[:, :])
            rel_sb = const.tile([128, T], F32)
            nc.sync.dma_start(out=rel_sb[:], in_=reli[:, :])
            wgt_sb = const.tile([128, T], F32)
            nc.sync.dma_start(out=wgt_sb[:], in_=wgti[:, :])
            iota_sb = const.tile([128, D], F32)
            nc.sync.dma_start(out=iota_sb[:], in_=iot[:, :])
            w1_sb = const.tile([128, D], F32)
            nc.sync.dma_start(out=w1_sb[:], in_=w1[:, :])
            w2_sb = const.tile([128, D], F32)
            nc.sync.dma_start(out=w2_sb[:], in_=w2[:, :])
            wa_sb = const.tile([128, KCL], F32)
            nc.sync.dma_start(out=wa_sb[:], in_=wa[:, :])
            b1r_sb = const.tile([128, D], F32)
            nc.sync.dma_start(out=b1r_sb[:], in_=b1r[:, :])
            b2r_sb = const.tile([128, D], F32)
            nc.sync.dma_start(out=b2r_sb[:], in_=b2r[:, :])
            bar_sb = const.tile([128, KCL], F32)
            nc.sync.dma_start(out=bar_sb[:], in_=bar[:, :])
            ident = const.tile([128, 128], F32)
            make_identity(nc, ident[:])
            ones_sb = const.tile([128, 1], F32)
            nc.vector.memset(ones_sb[:], 1.0)
            degw1_sb = const.tile([128, B], F32)

            h1_sb = const.tile([128, B, D], F32)
            h2_sb = const.tile([128, B, D], F32)

            qctr = [0]

            def layer(first, table, feats_own, W_sb, brep_sb, h_write,
                      dram_out):
                for g0 in range(0, B, GRP):
                    gb = min(GRP, B - g0)
                    w = gb * 128
                    q_ps = qpool.tile([128, GRP * 128], F32, tag="q")
                    bt = btpool.tile([128, GRP * 128], F32, tag="bt")
                    for j in range(gb):
                        b = g0 + j
                        gt = gpool.tile([128, TT, D], F32, tag="g")
                        # dma_gather calls above ~1024 indices hang the HW;
                        # split into <=8-tile (1024-idx) sub-calls, and
                        # alternate SWDGE queues so two Q7 descriptor
                        # generators run in parallel.
                        SUBT = 8
                        for (base, size, toff, tcnt) in halves:
                            for s0 in range(0, tcnt, SUBT):
                                sc = min(SUBT, tcnt - s0)
                                o = toff + s0
                                nc.gpsimd.dma_gather(
                                    out_ap=gt[:, o:o + sc, :],
                                    in_ap=table[base:base + size, :],
                                    idxs_ap=idx_sb[:, (b * TT + o) * 8:
                                                   (b * TT + o + sc) * 8],
                                    num_idxs=sc * 128,
                                    num_idxs_reg=sc * 128,
                                    elem_size=D,
                                    queue_num=qctr[0] % 2)
                                qctr[0] += 1
                        P = ppool.tile([128, 128], F32, tag="p")
                        Pd = None
                        if first:
                            Pd = pdpool.tile([128, 1], F32, tag="pd")
                        for t in range(TT):
                            tcol = b * TT + t
                            St = spool.tile([128, 128], F32, tag="s")
                            nc.vector.tensor_scalar(
                                out=St[:], in0=iota_sb[:],
                                scalar1=rel_sb[:, tcol:tcol + 1],
                                scalar2=wgt_sb[:, tcol:tcol + 1],
                                op0=OP.is_equal, op1=OP.mult)
                            nc.tensor.matmul(
                                P[:, 0:D], lhsT=St[:], rhs=gt[:, t, :],
                                start=(t == 0), stop=(t == TT - 1),
                                skip_group_check=True)
                            if first:
                                nc.tensor.matmul(
                                    Pd[:, 0:1], lhsT=St[:],
                                    rhs=ones_sb[:, 0:1],
                                    start=(t == 0), stop=(t == TT - 1),
                                    skip_group_check=True)
                        fo = feats_own(b)
                        Z = zpool.tile([128, D], F32, tag="z")
                        nc.vector.tensor_tensor(out=Z[:], in0=P[:, 0:D],
                                                in1=fo, op=OP.add)
                        if first:
                            nc.vector.tensor_scalar(
                                out=degw1_sb[:, b:b + 1], in0=Pd[:, 0:1],
                                scalar1=1.0, scalar2=None, op0=OP.add)
                        ZT = tpool.tile([128, 128], F32, tag="t")
                        nc.tensor.transpose(ZT[:], Z[:], ident[:])
                        ZTs = ztpool.tile([128, 128], F32, tag="zt")
                        nc.vector.tensor_copy(out=ZTs[:], in_=ZT[:])
                        nc.tensor.matmul(q_ps[:, j * 128:(j + 1) * 128],
                                         lhsT=ZTs[:], rhs=W_sb[:],
                                         start=True, stop=True)
                        nc.vector.tensor_scalar(
                            out=bt[:, j * 128:(j + 1) * 128], in0=brep_sb[:],
                            scalar1=degw1_sb[:, b:b + 1], scalar2=None,
                            op0=OP.mult)
                    # epilogue: q2 = q + bt ; h = selu(q2)
                    q2 = epool.tile([128, GRP * 128], F32, tag="q2")
                    nc.vector.tensor_tensor(out=q2[:, :w], in0=q_ps[:, :w],
                                            in1=bt[:, :w], op=OP.add)
                    m = epool.tile([128, GRP * 128], F32, tag="m")
                    nc.vector.tensor_scalar(out=m[:, :w], in0=q2[:, :w],
                                            scalar1=0.0, scalar2=None,
                                            op0=OP.min)
                    ex = epool.tile([128, GRP * 128], F32, tag="ex")
                    nc.scalar.activation(out=ex[:, :w], in_=m[:, :w],
                                         func=ACT.Exp)
                    nc.vector.tensor_scalar(out=ex[:, :w], in0=ex[:, :w],
                                            scalar1=SA, scalar2=SA,
                                            op0=OP.mult, op1=OP.subtract)
                    nc.vector.tensor_scalar(out=q2[:, :w], in0=q2[:, :w],
                                            scalar1=0.0, scalar2=SELU_SCALE,
                                            op0=OP.max, op1=OP.mult)
                    h_write(g0, gb, ex, q2)
                    if dram_out is not None:
                        dram_out(g0, gb)

            # ---------------- layer 1 ----------------
            def feats_own1(b):
                xo = xopool.tile([128, D], F32, tag="xo")
                nc.sync.dma_start(out=xo[:],
                                  in_=xown[b * 128:(b + 1) * 128, :])
                return xo[:]

            def h1_write(g0, gb, ex, q2):
                w = gb * 128
                nc.vector.tensor_tensor(
                    out=h1_sb[:, g0:g0 + gb, :],
                    in0=ex[:, :w].rearrange("p (g d) -> p g d", g=gb),
                    in1=q2[:, :w].rearrange("p (g d) -> p g d", g=gb),
                    op=OP.add)

            h1own_t = h1own[:, :].rearrange("(b p) r -> p b r", p=128)

            def h1_dram(g0, gb):
                nc.sync.dma_start(out=h1own_t[:, g0:g0 + gb, :],
                                  in_=h1_sb[:, g0:g0 + gb, :])

            layer(True, xfull, feats_own1, w1_sb, b1r_sb, h1_write, h1_dram)

            if no_collective:
                # timeline-sim stand-in keeping the dataflow dependency
                nc.sync.dma_start(out=h1full[0:NPC, :], in_=h1own[:, :])
            else:
                nc.gpsimd.collective_compute(
                    "AllGather", mybir.AluOpType.bypass,
                    replica_groups=[list(range(C))],
                    ins=[h1own[:, :]], outs=[h1full[:, :]])

            # ---------------- layer 2 ----------------
            def feats_own2(b):
                return h1_sb[:, b, :]

            def h2_write(g0, gb, ex, q2):
                w = gb * 128
                nc.vector.tensor_tensor(
                    out=h2_sb[:, g0:g0 + gb, :],
                    in0=ex[:, :w].rearrange("p (g d) -> p g d", g=gb),
                    in1=q2[:, :w].rearrange("p (g d) -> p g d", g=gb),
                    op=OP.add)

            layer(False, h1full, feats_own2, w2_sb, b2r_sb, h2_write, None)

            # ---------------- assignment head ----------------
            for b in range(B):
                ZT = tpool.tile([128, 128], F32, tag="t")
                nc.tensor.transpose(ZT[:], h2_sb[:, b, :], ident[:])
                ZTs = ztpool.tile([128, 128], F32, tag="zt")
                nc.vector.tensor_copy(out=ZTs[:], in_=ZT[:])
                za = qpool.tile([128, GRP * 128], F32, tag="q")
                nc.tensor.matmul(za[:, 0:KCL], lhsT=ZTs[:], rhs=wa_sb[:, :],
                                 start=True, stop=True)
                zs = l3pool.tile([128, KCL], F32, tag="zs")
                nc.vector.tensor_tensor(out=zs[:], in0=za[:, 0:KCL],
                                        in1=bar_sb[:], op=OP.add)
                mx = l3pool.tile([128, 1], F32, tag="mx")
                nc.vector.tensor_reduce(out=mx[:], in_=zs[:],
                                        axis=mybir.AxisListType.X, op=OP.max)
                nc.vector.tensor_scalar(out=zs[:], in0=zs[:],
                                        scalar1=mx[:, 0:1], scalar2=None,
                                        op0=OP.subtract)
                es = l3pool.tile([128, KCL], F32, tag="es")
                nc.scalar.activation(out=es[:], in_=zs[:], func=ACT.Exp)
                sm = l3pool.tile([128, 1], F32, tag="sm")
                nc.vector.tensor_reduce(out=sm[:], in_=es[:],
                                        axis=mybir.AxisListType.X, op=OP.add)
                rc = l3pool.tile([128, 1], F32, tag="rc")
                nc.vector.reciprocal(out=rc[:], in_=sm[:])
                oo = l3pool.tile([128, KCL], F32, tag="oo")
                nc.vector.tensor_scalar(out=oo[:], in0=es[:],
                                        scalar1=rc[:, 0:1], scalar2=None,
                                        op0=OP.mult)
                nc.sync.dma_start(out=outp[b * 128:(b + 1) * 128, :],
                                  in_=oo[:])

    nc.compile()
    return nc


# ----------------------------------------------------------------------------
# public entry point
# ----------------------------------------------------------------------------

def make_in_maps(meta, IDX, REL, WGT, xfull, xown, W1, b1, W2, b2, Wa, ba):
    iota = np.tile(np.arange(D, dtype=np.float32), (128, 1))
    b1r = np.tile(np.asarray(b1, np.float32), (128, 1))
    b2r = np.tile(np.asarray(b2, np.float32), (128, 1))
    bar = np.tile(np.asarray(ba, np.float32), (128, 1))
    in_maps = []
    for c in range(C):
        in_maps.append({
            "xfull": xfull, "xown": xown[c],
            "idx16": IDX[c], "reli": REL[c], "wgti": WGT[c],
            "w1": np.asarray(W1, np.float32), "w2": np.asarray(W2, np.float32),
            "wa": np.asarray(Wa, np.float32),
            "b1r": b1r, "b2r": b2r, "bar": bar, "iot": iota,
        })
    return in_maps


def get_compiled(x, edge_index, edge_weight):
    meta, IDX, REL, WGT, xfull, xown = preprocess(x, edge_index, edge_weight)
    key = (meta["B"], meta["TPB_LO"], meta["TPB_HI"], meta["NPC"],
           meta["Npad"])
    if key not in _CACHE:
        _CACHE[key] = build_program(*key)
    return _CACHE[key], meta, IDX, REL, WGT, xfull, xown


def kernel(x, edge_index, edge_weight, W1, b1, skip1, W2, b2, skip2, Wa, ba):
    x = np.asarray(x)
    edge_index = np.asarray(edge_index)
    edge_weight = np.asarray(edge_weight)
    if (x.shape[0] % C != 0 or x.shape[1] != D
            or x.shape[0] > 2 * HALF - 128 * C   # Npad must fit 2 halves
            or not np.allclose(np.asarray(skip1), 1.0)
            or not np.allclose(np.asarray(skip2), 1.0)):
        return _numpy_reference(x, edge_index, edge_weight, W1, b1, skip1,
                                W2, b2, skip2, Wa, ba)

    from concourse.bass_utils import run_bass_kernel_spmd

    nc, meta, IDX, REL, WGT, xfull, xown = get_compiled(
        x, edge_index, edge_weight)
    in_maps = make_in_maps(meta, IDX, REL, WGT, xfull, xown,
                           W1, b1, W2, b2, Wa, ba)
    res = run_bass_kernel_spmd(nc, in_maps, core_ids=list(range(C)))
    npc_real = meta["npc_real"]
    out = np.concatenate(
        [res.results[c]["assign"][:npc_real] for c in range(C)], axis=0)
    return np.ascontiguousarray(out.astype(np.float32))


# ----------------------------------------------------------------------------
# numpy fallback (also used to validate the device path in tests)
# ----------------------------------------------------------------------------

def _numpy_reference(x, edge_index, edge_weight, W1, b1, skip1, W2, b2,
                     skip2, Wa, ba):
    x = np.asarray(x, np.float64)
    dst, src = edge_index[0], edge_index[1]
    w = np.asarray(edge_weight, np.float64)

    def spmm(feats):
        out = np.zeros_like(feats)
        np.add.at(out, dst, feats[src] * w[:, None])
        return out

    def selu(v):
        return SELU_SCALE * np.where(v > 0, v, SELU_ALPHA * (np.exp(v) - 1))

    def gcn(feats, W, b, skip):
        t = feats @ np.asarray(W, np.float64) + np.asarray(b, np.float64)
        return selu(np.asarray(skip, np.float64) * t + spmm(t))

    h = gcn(x, W1, b1, skip1)
    h = gcn(h, W2, b2, skip2)
    z = h @ np.asarray(Wa, np.float64) + np.asarray(ba, np.float64)
    z = z - z.max(axis=1, keepdims=True)
    ez = np.exp(z)
    return (ez / ez.sum(axis=1, keepdims=True)).astype(np.float32)


# revision 19
# speedup vs baseline: 1.8638x; 1.4320x over previous
"""Distributed GCN (DMoN front-end) kernel for 8 Trainium2 NeuronCores.

Strategy (matches the sharding hint):
  - Nodes are partitioned contiguously across the 8 cores; each core owns its
    nodes' incident (incoming) edges, grouped by destination block of 128.
  - spmm is computed as a sequence of one-hot "segment matmuls": for each tile
    of 128 edges (sorted by destination), build S[e, d] = w_e * (rel_dst[e]==d)
    on the vector engine and accumulate P += S^T @ gathered_rows on the PE,
    where gathered_rows come from a dma_gather of source-node feature rows
    (one bulk gather per destination block and per index half, since
    dma_gather indices are int16).
  - The linear transform is applied AFTER aggregation (linearity):
        selu(skip*(xW+b) + spmm(xW+b)) == selu((x_own + spmm_raw(x))W +
        (deg_w + skip)*b)          (skip == 1 in this model)
    so the gather tables are the raw features (x, then h1) — no transformed
    table is ever materialized.
  - deg_w (sum of incident edge weights per node) is produced by a second
    PSUM accumulation chain (S^T @ ones) in layer 1 and reused in layer 2.
  - Between the two GCN layers, h1 is AllGathered across the 8 cores so every
    core can gather any source row of h1.
"""

import math

import numpy as np

C = 8            # cores
D = 128          # feature dim
KCL = 16         # clusters
GRP = 4          # dst blocks per PSUM epilogue group
HALF = 32768     # int16 index range per gather table slice
SELU_ALPHA = 1.6732632423543772
SELU_SCALE = 1.0507009873554805

_CACHE = {}


# ----------------------------------------------------------------------------
# host-side preprocessing (pure index manipulation + layout)
# ----------------------------------------------------------------------------

def preprocess(x, edge_index, edge_weight):
    N = x.shape[0]
    E = edge_index.shape[1]
    npc_real = N // C
    assert N % C == 0
    B = math.ceil(npc_real / 128)
    NPC = 128 * B
    Npad = C * NPC
    nhalf = 2 if Npad > HALF else 1
    assert Npad <= 2 * HALF

    dst = edge_index[0].astype(np.int64)
    src = edge_index[1].astype(np.int64)
    core = dst // npc_real
    dst_loc = dst - core * npc_real
    blk = dst_loc // 128
    rel = (dst_loc % 128).astype(np.float32)

    src_pad = (src // npc_real) * NPC + (src % npc_real)
    half = (src_pad >= HALF).astype(np.int64)

    gkey = (core * B + blk) * 2 + half
    counts = np.bincount(gkey, minlength=C * B * 2)
    c2 = counts.reshape(C * B, 2)
    TPB_LO = max(1, int(math.ceil(c2[:, 0].max() / 128)))
    TPB_HI = int(math.ceil(c2[:, 1].max() / 128)) if nhalf == 2 else 0
    TT = TPB_LO + TPB_HI
    T = B * TT

    order = np.argsort(gkey, kind="stable")
    starts = np.zeros(C * B * 2 + 1, np.int64)
    starts[1:] = np.cumsum(counts)
    pos = np.arange(E, dtype=np.int64) - starts[gkey[order]]

    core_s = core[order]
    blk_s = blk[order]
    half_s = half[order]
    col = blk_s * TT + half_s * TPB_LO + pos // 128
    row = pos % 128

    REL = np.zeros((C, 128, T), np.float32)
    WGT = np.zeros((C, 128, T), np.float32)
    REL[core_s, row, col] = rel[order]
    WGT[core_s, row, col] = np.asarray(edge_weight, np.float32)[order]

    # int16 gather indices in dma_gather's wrapped layout:
    # within one call, edge slot q maps to idx16[q % 16, c0 + q // 16]; the
    # 16-row pattern is replicated to all 128 partitions (one copy per Q7
    # core).
    IDX = np.zeros((C, 16, T * 8), np.int16)
    relsrc = (src_pad[order] - half_s * HALF).astype(np.int16)
    q = pos
    c0 = (blk_s * TT + half_s * TPB_LO) * 8
    IDX[core_s, q % 16, c0 + q // 16] = relsrc
    IDX = np.tile(IDX, (1, 8, 1))               # [C, 128, T*8]

    # padded gather table for layer 1
    xt = np.zeros((C, NPC, D), np.float32)
    xt[:, :npc_real, :] = np.asarray(x, np.float32).reshape(C, npc_real, D)
    xfull = np.ascontiguousarray(xt.reshape(Npad, D))

    meta = dict(N=N, E=E, npc_real=npc_real, B=B, NPC=NPC, Npad=Npad,
                TPB_LO=TPB_LO, TPB_HI=TPB_HI, TT=TT, T=T)
    return meta, IDX, REL, WGT, xfull, xt


# ----------------------------------------------------------------------------
# bass program
# ----------------------------------------------------------------------------

def build_program(B, TPB_LO, TPB_HI, NPC, Npad, no_collective=False):
    import concourse.bass as bass
    import concourse.bacc as bacc
    import concourse.mybir as mybir
    from concourse.tile import TileContext
    from concourse.masks import make_identity

    F32 = mybir.dt.float32
    I16 = mybir.dt.int16
    OP = mybir.AluOpType
    ACT = mybir.ActivationFunctionType
    TT = TPB_LO + TPB_HI
    T = B * TT
    SA = SELU_SCALE * SELU_ALPHA
    halves = [(0, min(Npad, HALF), 0, TPB_LO)]
    if TPB_HI:
        halves.append((HALF, Npad - HALF, TPB_LO, TPB_HI))

    nc = bacc.Bacc("TRN2", target_bir_lowering=False, debug=False,
                   num_devices=C, num_swdge_queues=3)

    xfull = nc.dram_tensor("xfull", [Npad, D], F32, kind="ExternalInput")
    xown = nc.dram_tensor("xown", [NPC, D], F32, kind="ExternalInput")
    idx16 = nc.dram_tensor("idx16", [128, T * 8], I16, kind="ExternalInput")
    reli = nc.dram_tensor("reli", [128, T], F32, kind="ExternalInput")
    wgti = nc.dram_tensor("wgti", [128, T], F32, kind="ExternalInput")
    w1 = nc.dram_tensor("w1", [D, D], F32, kind="ExternalInput")
    w2 = nc.dram_tensor("w2", [D, D], F32, kind="ExternalInput")
    wa = nc.dram_tensor("wa", [D, KCL], F32, kind="ExternalInput")
    b1r = nc.dram_tensor("b1r", [128, D], F32, kind="ExternalInput")
    b2r = nc.dram_tensor("b2r", [128, D], F32, kind="ExternalInput")
    bar = nc.dram_tensor("bar", [128, KCL], F32, kind="ExternalInput")
    iot = nc.dram_tensor("iot", [128, D], F32, kind="ExternalInput")

    h1own = nc.dram_tensor("h1own", [NPC, D], F32)
    h1full = nc.dram_tensor("h1full", [Npad, D], F32, addr_space="Shared")
    outp = nc.dram_tensor("assign", [NPC, KCL], F32, kind="ExternalOutput")

    with TileContext(nc) as tc:
        with tc.tile_pool(name="const", bufs=1) as const, \
             tc.tile_pool(name="g", bufs=3) as gpool, \
             tc.tile_pool(name="s", bufs=8) as spool, \
             tc.tile_pool(name="z", bufs=4) as zpool, \
             tc.tile_pool(name="zt", bufs=4) as ztpool, \
             tc.tile_pool(name="bt", bufs=2) as btpool, \
             tc.tile_pool(name="ep", bufs=2) as epool, \
             tc.tile_pool(name="xo", bufs=4) as xopool, \
             tc.tile_pool(name="l3", bufs=3) as l3pool, \
             tc.tile_pool(name="pp", bufs=2, space="PSUM") as ppool, \
             tc.tile_pool(name="pd", bufs=2, space="PSUM") as pdpool, \
             tc.tile_pool(name="qp", bufs=2, space="PSUM") as qpool, \
             tc.tile_pool(name="tp", bufs=2, space="PSUM") as tpool:

            # ---- persistent state ----
            idx_sb = const.tile([128, T * 8], I16)
            nc.sync.dma_start(out=idx_sb[:], in_=idx16[:, :])
            rel_sb = const.tile([128, T], F32)
            nc.sync.dma_start(out=rel_sb[:], in_=reli[:, :])
            wgt_sb = const.tile([128, T], F32)
            nc.sync.dma_start(out=wgt_sb[:], in_=wgti[:, :])
            iota_sb = const.tile([128, D], F32)
            nc.sync.dma_start(out=iota_sb[:], in_=iot[:, :])
            w1_sb = const.tile([128, D], F32)
            nc.sync.dma_start(out=w1_sb[:], in_=w1[:, :])
            w2_sb = const.tile([128, D], F32)
            nc.sync.dma_start(out=w2_sb[:], in_=w2[:, :])
            wa_sb = const.tile([128, KCL], F32)
            nc.sync.dma_start(out=wa_sb[:], in_=wa[:, :])
            b1r_sb = const.tile([128, D], F32)
            nc.sync.dma_start(out=b1r_sb[:], in_=b1r[:, :])
            b2r_sb = const.tile([128, D], F32)
            nc.sync.dma_start(out=b2r_sb[:], in_=b2r[:, :])
            bar_sb = const.tile([128, KCL], F32)
            nc.sync.dma_start(out=bar_sb[:], in_=bar[:, :])
            ident = const.tile([128, 128], F32)
            make_identity(nc, ident[:])
            ones_sb = const.tile([128, 1], F32)
            nc.vector.memset(ones_sb[:], 1.0)
            degw1_sb = const.tile([128, B], F32)

            h1_sb = const.tile([128, B, D], F32)
            h2_sb = const.tile([128, B, D], F32)

            qctr = [0]

            def layer(first, table, feats_own, W_sb, brep_sb, h_write,
                      dram_out):
                for g0 in range(0, B, GRP):
                    gb = min(GRP, B - g0)
                    w = gb * 128
                    q_ps = qpool.tile([128, GRP * 128], F32, tag="q")
                    bt = btpool.tile([128, GRP * 128], F32, tag="bt")
                    for j in range(gb):
                        b = g0 + j
                        gt = gpool.tile([128, TT, D], F32, tag="g")
                        # dma_gather calls above ~1024 indices hang the HW;
                        # split into <=8-tile (1024-idx) sub-calls, and
                        # alternate SWDGE queues so two Q7 descriptor
                        # generators run in parallel.
                        SUBT = 8
                        for (base, size, toff, tcnt) in halves:
                            for s0 in range(0, tcnt, SUBT):
                                sc = min(SUBT, tcnt - s0)
                                o = toff + s0
                                nc.gpsimd.dma_gather(
                                    out_ap=gt[:, o:o + sc, :],
                                    in_ap=table[base:base + size, :],
                                    idxs_ap=idx_sb[:, (b * TT + o) * 8:
                                                   (b * TT + o + sc) * 8],
                                    num_idxs=sc * 128,
                                    num_idxs_reg=sc * 128,
                                    elem_size=D,
                                    queue_num=qctr[0] % 3)
                                qctr[0] += 1
                        P = ppool.tile([128, 128], F32, tag="p")
                        Pd = None
                        if first:
                            Pd = pdpool.tile([128, 1], F32, tag="pd")
                        for t in range(TT):
                            tcol = b * TT + t
                            St = spool.tile([128, 128], F32, tag="s")
                            nc.vector.tensor_scalar(
                                out=St[:], in0=iota_sb[:],
                                scalar1=rel_sb[:, tcol:tcol + 1],
                                scalar2=wgt_sb[:, tcol:tcol + 1],
                                op0=OP.is_equal, op1=OP.mult)
                            nc.tensor.matmul(
                                P[:, 0:D], lhsT=St[:], rhs=gt[:, t, :],
                                start=(t == 0), stop=(t == TT - 1),
                                skip_group_check=True)
                            if first:
                                nc.tensor.matmul(
                                    Pd[:, 0:1], lhsT=St[:],
                                    rhs=ones_sb[:, 0:1],
                                    start=(t == 0), stop=(t == TT - 1),
                                    skip_group_check=True)
                        fo = feats_own(b)
                        Z = zpool.tile([128, D], F32, tag="z")
                        nc.vector.tensor_tensor(out=Z[:], in0=P[:, 0:D],
                                                in1=fo, op=OP.add)
                        if first:
                            nc.vector.tensor_scalar(
                                out=degw1_sb[:, b:b + 1], in0=Pd[:, 0:1],
                                scalar1=1.0, scalar2=None, op0=OP.add)
                        ZT = tpool.tile([128, 128], F32, tag="t")
                        nc.tensor.transpose(ZT[:], Z[:], ident[:])
                        ZTs = ztpool.tile([128, 128], F32, tag="zt")
                        nc.vector.tensor_copy(out=ZTs[:], in_=ZT[:])
                        nc.tensor.matmul(q_ps[:, j * 128:(j + 1) * 128],
                                         lhsT=ZTs[:], rhs=W_sb[:],
                                         start=True, stop=True)
                        nc.vector.tensor_scalar(
                            out=bt[:, j * 128:(j + 1) * 128], in0=brep_sb[:],
                            scalar1=degw1_sb[:, b:b + 1], scalar2=None,
                            op0=OP.mult)
                    # epilogue: q2 = q + bt ; h = selu(q2)
                    q2 = epool.tile([128, GRP * 128], F32, tag="q2")
                    nc.vector.tensor_tensor(out=q2[:, :w], in0=q_ps[:, :w],
                                            in1=bt[:, :w], op=OP.add)
                    m = epool.tile([128, GRP * 128], F32, tag="m")
                    nc.vector.tensor_scalar(out=m[:, :w], in0=q2[:, :w],
                                            scalar1=0.0, scalar2=None,
                                            op0=OP.min)
                    ex = epool.tile([128, GRP * 128], F32, tag="ex")
                    nc.scalar.activation(out=ex[:, :w], in_=m[:, :w],
                                         func=ACT.Exp)
                    nc.vector.tensor_scalar(out=ex[:, :w], in0=ex[:, :w],
                                            scalar1=SA, scalar2=SA,
                                            op0=OP.mult, op1=OP.subtract)
                    nc.vector.tensor_scalar(out=q2[:, :w], in0=q2[:, :w],
                                            scalar1=0.0, scalar2=SELU_SCALE,
                                            op0=OP.max, op1=OP.mult)
                    h_write(g0, gb, ex, q2)
                    if dram_out is not None:
                        dram_out(g0, gb)

            # ---------------- layer 1 ----------------
            def feats_own1(b):
                xo = xopool.tile([128, D], F32, tag="xo")
                nc.sync.dma_start(out=xo[:],
                                  in_=xown[b * 128:(b + 1) * 128, :])
                return xo[:]

            def h1_write(g0, gb, ex, q2):
                w = gb * 128
                nc.vector.tensor_tensor(
                    out=h1_sb[:, g0:g0 + gb, :],
                    in0=ex[:, :w].rearrange("p (g d) -> p g d", g=gb),
                    in1=q2[:, :w].rearrange("p (g d) -> p g d", g=gb),
                    op=OP.add)

            h1own_t = h1own[:, :].rearrange("(b p) r -> p b r", p=128)

            def h1_dram(g0, gb):
                nc.sync.dma_start(out=h1own_t[:, g0:g0 + gb, :],
                                  in_=h1_sb[:, g0:g0 + gb, :])

            layer(True, xfull, feats_own1, w1_sb, b1r_sb, h1_write, h1_dram)

            if no_collective:
                # timeline-sim stand-in keeping the dataflow dependency
                nc.sync.dma_start(out=h1full[0:NPC, :], in_=h1own[:, :])
            else:
                nc.gpsimd.collective_compute(
                    "AllGather", mybir.AluOpType.bypass,
                    replica_groups=[list(range(C))],
                    ins=[h1own[:, :]], outs=[h1full[:, :]])

            # ---------------- layer 2 ----------------
            def feats_own2(b):
                return h1_sb[:, b, :]

            def h2_write(g0, gb, ex, q2):
                w = gb * 128
                nc.vector.tensor_tensor(
                    out=h2_sb[:, g0:g0 + gb, :],
                    in0=ex[:, :w].rearrange("p (g d) -> p g d", g=gb),
                    in1=q2[:, :w].rearrange("p (g d) -> p g d", g=gb),
                    op=OP.add)

            layer(False, h1full, feats_own2, w2_sb, b2r_sb, h2_write, None)

            # ---------------- assignment head ----------------
            for b in range(B):
                ZT = tpool.tile([128, 128], F32, tag="t")
                nc.tensor.transpose(ZT[:], h2_sb[:, b, :], ident[:])
                ZTs = ztpool.tile([128, 128], F32, tag="zt")
                nc.vector.tensor_copy(out=ZTs[:], in_=ZT[:])
                za = qpool.tile([128, GRP * 128], F32, tag="q")
                nc.tensor.matmul(za[:, 0:KCL], lhsT=ZTs[:], rhs=wa_sb[:, :],
                                 start=True, stop=True)
                zs = l3pool.tile([128, KCL], F32, tag="zs")
                nc.vector.tensor_tensor(out=zs[:], in0=za[:, 0:KCL],
                                        in1=bar_sb[:], op=OP.add)
                mx = l3pool.tile([128, 1], F32, tag="mx")
                nc.vector.tensor_reduce(out=mx[:], in_=zs[:],
                                        axis=mybir.AxisListType.X, op=OP.max)
                nc.vector.tensor_scalar(out=zs[:], in0=zs[:],
                                        scalar1=mx[:, 0:1], scalar2=None,
                                        op0=OP.subtract)
                es = l3pool.tile([128, KCL], F32, tag="es")
                nc.scalar.activation(out=es[:], in_=zs[:], func=ACT.Exp)
                sm = l3pool.tile([128, 1], F32, tag="sm")
                nc.vector.tensor_reduce(out=sm[:], in_=es[:],
                                        axis=mybir.AxisListType.X, op=OP.add)
                rc = l3pool.tile([128, 1], F32, tag="rc")
                nc.vector.reciprocal(out=rc[:], in_=sm[:])
                oo = l3pool.tile([128, KCL], F32, tag="oo")
                nc.vector.tensor_scalar(out=oo[:], in0=es[:],
                                        scalar1=rc[:, 0:1], scalar2=None,
                                        op0=OP.mult)
                nc.sync.dma_start(out=outp[b * 128:(b + 1) * 128, :],
                                  in_=oo[:])

    nc.compile()
    return nc


# ----------------------------------------------------------------------------
# public entry point
# ----------------------------------------------------------------------------

def make_in_maps(meta, IDX, REL, WGT, xfull, xown, W1, b1, W2, b2, Wa, ba):
    iota = np.tile(np.arange(D, dtype=np.float32), (128, 1))
    b1r = np.tile(np.asarray(b1, np.float32), (128, 1))
    b2r = np.tile(np.asarray(b2, np.float32), (128, 1))
    bar = np.tile(np.asarray(ba, np.float32), (128, 1))
    in_maps = []
    for c in range(C):
        in_maps.append({
            "xfull": xfull, "xown": xown[c],
            "idx16": IDX[c], "reli": REL[c], "wgti": WGT[c],
            "w1": np.asarray(W1, np.float32), "w2": np.asarray(W2, np.float32),
            "wa": np.asarray(Wa, np.float32),
            "b1r": b1r, "b2r": b2r, "bar": bar, "iot": iota,
        })
    return in_maps


def get_compiled(x, edge_index, edge_weight):
    meta, IDX, REL, WGT, xfull, xown = preprocess(x, edge_index, edge_weight)
    key = (meta["B"], meta["TPB_LO"], meta["TPB_HI"], meta["NPC"],
           meta["Npad"])
    if key not in _CACHE:
        _CACHE[key] = build_program(*key)
    return _CACHE[key], meta, IDX, REL, WGT, xfull, xown


def kernel(x, edge_index, edge_weight, W1, b1, skip1, W2, b2, skip2, Wa, ba):
    x = np.asarray(x)
    edge_index = np.asarray(edge_index)
    edge_weight = np.asarray(edge_weight)
    if (x.shape[0] % C != 0 or x.shape[1] != D
            or x.shape[0] > 2 * HALF - 128 * C   # Npad must fit 2 halves
            or not np.allclose(np.asarray(skip1), 1.0)
            or not np.allclose(np.asarray(skip2), 1.0)):
        return _numpy_reference(x, edge_index, edge_weight, W1, b1, skip1,
                                W2, b2, skip2, Wa, ba)

    from concourse.bass_utils import run_bass_kernel_spmd

    nc, meta, IDX, REL, WGT, xfull, xown = get_compiled(
        x, edge_index, edge_weight)
    in_maps = make_in_maps(meta, IDX, REL, WGT, xfull, xown,
                           W1, b1, W2, b2, Wa, ba)
    res = run_bass_kernel_spmd(nc, in_maps, core_ids=list(range(C)))
    npc_real = meta["npc_real"]
    out = np.concatenate(
        [res.results[c]["assign"][:npc_real] for c in range(C)], axis=0)
    return np.ascontiguousarray(out.astype(np.float32))


# ----------------------------------------------------------------------------
# numpy fallback (also used to validate the device path in tests)
# ----------------------------------------------------------------------------

def _numpy_reference(x, edge_index, edge_weight, W1, b1, skip1, W2, b2,
                     skip2, Wa, ba):
    x = np.asarray(x, np.float64)
    dst, src = edge_index[0], edge_index[1]
    w = np.asarray(edge_weight, np.float64)

    def spmm(feats):
        out = np.zeros_like(feats)
        np.add.at(out, dst, feats[src] * w[:, None])
        return out

    def selu(v):
        return SELU_SCALE * np.where(v > 0, v, SELU_ALPHA * (np.exp(v) - 1))

    def gcn(feats, W, b, skip):
        t = feats @ np.asarray(W, np.float64) + np.asarray(b, np.float64)
        return selu(np.asarray(skip, np.float64) * t + spmm(t))

    h = gcn(x, W1, b1, skip1)
    h = gcn(h, W2, b2, skip2)
    z = h @ np.asarray(Wa, np.float64) + np.asarray(ba, np.float64)
    z = z - z.max(axis=1, keepdims=True)
    ez = np.exp(z)
    return (ez / ez.sum(axis=1, keepdims=True)).astype(np.float32)
